# revision 25
# baseline (speedup 1.0000x reference)
"""Trainium2 Bass kernel for nn_RelativeMultiHeadAttention.

Full (unsharded) numpy inputs in, full output out. Internally shards across
8 NeuronCores: core c handles batch b = c//4 and head pair hp = c%4
(heads 2*hp, 2*hp+1).

v2: q-chunk software pipeline. The relative shift is q-local, so the kernel
processes queries in 4 chunks of 512:
  - pos scores stream to a padded DRAM buffer G (Transformer-XL shift becomes
    an overlapping-window read of G); shifted scores for chunk c need only
    G rows [512c, 512c+512], so the write->transposed-read roundtrip is
    pipelined chunk by chunk instead of a full barrier.
  - one batched xbar-transpose DMA per (chunk, head) pulls shifted_T
    [128, 16, 512] (out[p, kt, q] = G[row q, col 128*kt+p]).
  - content scores computed transposed; DVE/Pool add shifted, ACT exps to
    bf16 attn (no max-subtraction: |logit*scale| < ~4); A.V accumulates
    ctx_T[65, 512] per head with an appended ones-row giving Z = sum_k attn.
  - out projection per q-tile with per-partition 1/Z scaling; per-core
    partial written bf16; host sums partials + exact bv/bo correction.
SCALE is folded into Wq/bq on the host so logits come out pre-scaled.
"""

import numpy as np
import ml_dtypes

B, L, D, H = 2, 2048, 512, 8
DH = D // H            # 64
HPC = 2                # heads per core
NCORES = 8
SCALE = 1.0 / float(np.sqrt(D))
LQT = L // 128         # 16 q/k tiles of 128
NCH = 4                # q chunks
CQ = L // NCH          # 512 queries per chunk
GROWS = L + 1          # padded G row length (2049)
WARM = 7               # pos tiles computed before chunk 0

_BF16 = ml_dtypes.bfloat16


def build_nc():
    import concourse.bass as bass
    import concourse.mybir as mybir
    from concourse.bacc import Bacc
    from concourse.tile import TileContext
    from contextlib import ExitStack

    bf16 = mybir.dt.bfloat16
    f32 = mybir.dt.float32
    AF = mybir.ActivationFunctionType
    ALU = mybir.AluOpType

    nc = Bacc()

    xq = nc.declare_dram_parameter("xq_t", [D, L], bf16, isOutput=False)
    xk = nc.declare_dram_parameter("xk_t", [D, L], bf16, isOutput=False)
    xp = nc.declare_dram_parameter("xp_t", [D, L], bf16, isOutput=False)
    xv = nc.declare_dram_parameter("xv_t", [D, L], bf16, isOutput=False)
    wq = nc.declare_dram_parameter("wq", [D, HPC * DH], bf16, isOutput=False)
    wk = nc.declare_dram_parameter("wk", [D, HPC * DH], bf16, isOutput=False)
    wp = nc.declare_dram_parameter("wp", [D, HPC * DH], bf16, isOutput=False)
    wv = nc.declare_dram_parameter("wv", [D, HPC * DH], bf16, isOutput=False)
    wo = nc.declare_dram_parameter("wo", [HPC * DH, D], bf16, isOutput=False)
    ident = nc.declare_dram_parameter("ident", [128, 128], bf16, isOutput=False)
    bq = nc.declare_dram_parameter("bq", [HPC * DH, 1], f32, isOutput=False)
    bk = nc.declare_dram_parameter("bk", [HPC * DH, 1], f32, isOutput=False)
    out = nc.declare_dram_parameter("out", [L * D], bf16, isOutput=True)

    # per-(chunk, head) G buffers: 513 rows each (chunk boundary row is
    # written twice) so the transposed read of chunk c depends only on its
    # own chunk's writers (DRAM dep tracking is per-tensor).
    gs = [[nc.dram_tensor(f"g{h}_{c}", [(513 if c < NCH - 1 else 512) * GROWS],
                          bf16) for c in range(NCH)] for h in range(HPC)]

    with TileContext(nc) as tc, ExitStack() as top:
        # ---------------- pools ----------------
        persist = top.enter_context(tc.tile_pool(name="persist", bufs=1))

        def ptile(shape, dtype, name):
            return persist.tile(shape, dtype, name=name, tag=name)

        qT = ptile([128, L], bf16, "qT")
        kT = ptile([128, L], bf16, "kT")
        pT = ptile([128, L], bf16, "pT")
        vaug = ptile([128, LQT, 2 * (DH + 1)], bf16, "vaug")
        wo_sb = ptile([128, D], bf16, "wo_sb")
        ctxT = ptile([128, L], bf16, "ctxT")
        bq_sb = ptile([128, 1], f32, "bq_sb")
        bk_sb = ptile([128, 1], f32, "bk_sb")
        ones_sb = ptile([1, 1], f32, "ones_sb")
        zrow0 = ptile([1, L], f32, "zrow0")
        zrow1 = ptile([1, L], f32, "zrow1")
        rz0 = ptile([128, LQT], f32, "rz0")
        rz1 = ptile([128, LQT], f32, "rz1")
        id_sb = ptile([128, 128], bf16, "id_sb")
        zrows = [zrow0, zrow1]
        rzs = [rz0, rz1]

        # single PSUM pool, 8 banks total across tags:
        # score(2) + pos(2) + po(2) + av0(1) + av1(1)
        pp = top.enter_context(tc.tile_pool(name="pp", bufs=1, space="PSUM"))

        nc.vector.memset(ones_sb[:, :], 1.0)
        nc.vector.memset(vaug[:, :, DH:DH + 1], 1.0)
        nc.vector.memset(vaug[:, :, 2 * DH + 1:2 * DH + 2], 1.0)
        nc.scalar.dma_start(out=wo_sb[:, :], in_=wo[:, :])
        nc.scalar.dma_start(out=bq_sb[:, :], in_=bq[:, :])
        nc.scalar.dma_start(out=bk_sb[:, :], in_=bk[:, :])
        nc.scalar.dma_start(out=id_sb[:, :], in_=ident[:, :])

        # ---------------- phase 1: projections ----------------
        with ExitStack() as p1:
            inpool = p1.enter_context(tc.tile_pool(name="inpool", bufs=1))
            x_sbs = {}
            w_sbs = {}
            for name, src in (("p", xp), ("q", xq), ("k", xk), ("v", xv)):
                t = inpool.tile([128, 4, L], bf16, name=f"x_{name}", tag=f"x_{name}")
                nc.sync.dma_start(
                    out=t[:, :, :],
                    in_=src[:, :].rearrange("(c p) m -> p c m", p=128))
                x_sbs[name] = t
            for name, src in (("p", wp), ("q", wq), ("k", wk), ("v", wv)):
                t = inpool.tile([128, 4, HPC * DH], bf16, name=f"w_{name}",
                                tag=f"w_{name}")
                nc.scalar.dma_start(
                    out=t[:, :, :],
                    in_=src[:, :].rearrange("(c p) m -> p c m", p=128))
                w_sbs[name] = t

            # pT/qT/kT: [128 (2 heads x 64ch), L], bias folded; q pre-scaled
            # on host (SCALE into wq/bq) so scores come out pre-scaled.
            for name, dst, bias in (
                    ("p", pT, None),
                    ("q", qT, bq_sb),
                    ("k", kT, bk_sb)):
                xs, ws = x_sbs[name], w_sbs[name]
                for n in range(4):
                    ps = pp.tile([128, 512], f32, name="ps_pj", tag="score",
                                 bufs=2)
                    for ck in range(4):
                        nc.tensor.matmul(
                            ps[:, :], lhsT=ws[:, ck, :],
                            rhs=xs[:, ck, n * 512:(n + 1) * 512],
                            start=(ck == 0), stop=(ck == 3))
                    o = dst[:, n * 512:(n + 1) * 512]
                    if name == "k":
                        nc.vector.tensor_scalar_add(o, ps[:, :], bias[:, 0:1])
                    elif name == "q":
                        nc.scalar.add(o, ps[:, :], bias[:, 0:1])
                    else:
                        nc.scalar.copy(o, ps[:, :])

            # v natural: [L, 128ch] -> vaug [128, t, [v0|1|v1|1]]
            xs, ws = x_sbs["v"], w_sbs["v"]
            for t in range(LQT):
                ps = pp.tile([128, 512], f32, name="ps_v", tag="px", bufs=2)
                for ck in range(4):
                    nc.tensor.matmul(
                        ps[:, 0:128], lhsT=xs[:, ck, t * 128:(t + 1) * 128],
                        rhs=ws[:, ck, :], start=(ck == 0), stop=(ck == 3))
                nc.vector.tensor_copy(vaug[:, t, 0:DH], ps[:, 0:DH])
                nc.vector.tensor_copy(vaug[:, t, DH + 1:2 * DH + 1],
                                      ps[:, DH:2 * DH])

        # ---------------- phase 2: pipelined pos/shift/attn ----------------
        stpool = top.enter_context(tc.tile_pool(name="stpool", bufs=4))
        shpool = top.enter_context(tc.tile_pool(name="shpool", bufs=3))
        wpool = top.enter_context(tc.tile_pool(name="wpool", bufs=1))

        stage_tiles = {}
        sh_tiles = {}
        def pos_piece(t, h, q4, warm):
            hb = h * DH
            if q4 == 0:
                st = stpool.tile([128, GROWS], bf16, name=f"st{t}_{h}",
                                 tag="st")
                stage_tiles[(t, h)] = st
                nc.gpsimd.memset(st[:, L:GROWS], 0.0)
            st = stage_tiles[(t, h)]
            ps = pp.tile([128, 512], f32, name="ps_pos", tag="px", bufs=2)
            nc.tensor.matmul(
                ps[:, :], lhsT=qT[hb:hb + DH, t * 128:(t + 1) * 128],
                rhs=pT[hb:hb + DH, q4 * 512:(q4 + 1) * 512],
                start=True, stop=True)
            o = st[:, q4 * 512:(q4 + 1) * 512]
            # steady-state copies on DVE (ACT is exp-bound); warmup alternates
            if warm and q4 % 2 == 0:
                nc.scalar.copy(o, ps[:, :])
            else:
                nc.vector.tensor_copy(o, ps[:, :])
            if q4 == 3:
                cc = t // 4
                nc.gpsimd.dma_start(
                    out=bass.AP(gs[h][cc], (t % 4) * 128 * GROWS,
                                [[GROWS, 128], [1, GROWS]]),
                    in_=st[:, :])
                if t % 4 == 0 and t > 0:
                    # chunk boundary row: first row of this tile is also the
                    # last (513th) row of the previous chunk's G
                    nc.gpsimd.dma_start(
                        out=bass.AP(gs[h][cc - 1], 512 * GROWS,
                                    [[GROWS, 1], [1, GROWS]]),
                        in_=st[0:1, :])
                del stage_tiles[(t, h)]

        def issue_T(c):
            for h in range(HPC):
                sht = shpool.tile([128, LQT, CQ], bf16, name=f"sh{c}_{h}",
                                  tag=f"sh{h}")
                sh_tiles[(c, h)] = sht
                for half in range(2):
                    nc.sync.dma_start(
                        out=sht[:, 8 * half:8 * half + 8, :],
                        in_=bass.AP(gs[h][c],
                                    (L - 1) - CQ * c + 1024 * half,
                                    [[L, CQ], [1, 1024]]),
                        transpose=True)

        pieces = [(t, h, q4) for t in range(LQT) for h in range(HPC)
                  for q4 in range(4)]
        warm_n = WARM * HPC * 4
        for pc in pieces[:warm_n]:
            pos_piece(*pc, warm=True)
        rest = pieces[warm_n:]
        ri = 0

        issue_T(0)

        avs = [None, None]
        NKP = LQT // 2  # kt pairs
        for c in range(NCH):
            prev = [None, None]  # previous pair's attn pending A.V
            for ktp in range(NKP):
                if ktp == 4 and c + 1 < NCH:
                    issue_T(c + 1)
                k0, k1 = 2 * ktp, 2 * ktp + 1
                for h in range(HPC):
                    for _ in range(2):
                        if ri < len(rest):
                            pos_piece(*rest[ri], warm=False)
                            ri += 1
                    hb = h * DH
                    # kt-pair content into a 2-bank tile, shifted scores
                    # accumulated by PE identity matmuls, one big exp
                    ct = pp.tile([128, 2 * CQ], f32, name="ps_ct", tag="score",
                                 bufs=2)
                    nc.tensor.matmul(
                        ct[:, 0:CQ],
                        lhsT=kT[hb:hb + DH, k0 * 128:(k0 + 1) * 128],
                        rhs=qT[hb:hb + DH, c * CQ:(c + 1) * CQ],
                        start=True, stop=False)
                    nc.tensor.matmul(
                        ct[:, CQ:2 * CQ],
                        lhsT=kT[hb:hb + DH, k1 * 128:(k1 + 1) * 128],
                        rhs=qT[hb:hb + DH, c * CQ:(c + 1) * CQ],
                        start=True, stop=False)
                    nc.tensor.matmul(ct[:, 0:CQ], lhsT=id_sb[:, :],
                                     rhs=sh_tiles[(c, h)][:, k0, :],
                                     start=False, stop=True)
                    nc.tensor.matmul(ct[:, CQ:2 * CQ], lhsT=id_sb[:, :],
                                     rhs=sh_tiles[(c, h)][:, k1, :],
                                     start=False, stop=True)
                    # A.V for previous pair while ACT exps this one
                    if prev[h] is not None:
                        pk, pattn = prev[h]
                        nc.tensor.matmul(
                            avs[h][:, :],
                            lhsT=vaug[:, 2 * pk, h * (DH + 1):(h + 1) * (DH + 1)],
                            rhs=pattn[:, 0:CQ],
                            start=(pk == 0), stop=False)
                        nc.tensor.matmul(
                            avs[h][:, :],
                            lhsT=vaug[:, 2 * pk + 1, h * (DH + 1):(h + 1) * (DH + 1)],
                            rhs=pattn[:, CQ:2 * CQ],
                            start=False, stop=False)
                    attn = wpool.tile([128, 2 * CQ], bf16, name="attn",
                                      tag="attn", bufs=3)
                    nc.scalar.activation(attn[:, :], ct[:, :], AF.Exp,
                                         bias=0.0, scale=1.0)
                    if ktp == 0:
                        avs[h] = pp.tile([DH + 1, CQ], f32, name=f"av{h}",
                                         tag=f"av{h}", bufs=1)
                    prev[h] = (ktp, attn)
            for h in range(HPC):
                pk, pattn = prev[h]
                nc.tensor.matmul(
                    avs[h][:, :],
                    lhsT=vaug[:, 2 * pk, h * (DH + 1):(h + 1) * (DH + 1)],
                    rhs=pattn[:, 0:CQ], start=False, stop=False)
                nc.tensor.matmul(
                    avs[h][:, :],
                    lhsT=vaug[:, 2 * pk + 1, h * (DH + 1):(h + 1) * (DH + 1)],
                    rhs=pattn[:, CQ:2 * CQ], start=False, stop=True)
            del sh_tiles[(c, 0)], sh_tiles[(c, 1)]

            # ctx rows + Z row out of the A.V accumulators
            for h in range(HPC):
                nc.vector.tensor_copy(
                    ctxT[64 * h:64 * h + DH, c * CQ:(c + 1) * CQ],
                    avs[h][0:DH, :])
                nc.scalar.copy(zrows[h][0:1, c * CQ:(c + 1) * CQ],
                               avs[h][DH:DH + 1, :])

            # Z -> partitions (tiny PE transposes), then reciprocal
            zt = pp.tile([128, 512], f32, name="zt", tag="px", bufs=2)
            for i in range(4):
                t = 4 * c + i
                nc.tensor.matmul(zt[:, i:i + 1],
                                 lhsT=zrow0[0:1, t * 128:(t + 1) * 128],
                                 rhs=ones_sb[0:1, 0:1], start=True, stop=True)
                nc.tensor.matmul(zt[:, 4 + i:5 + i],
                                 lhsT=zrow1[0:1, t * 128:(t + 1) * 128],
                                 rhs=ones_sb[0:1, 0:1], start=True, stop=True)
            nc.vector.reciprocal(rz0[:, 4 * c:4 * c + 4], zt[:, 0:4])
            nc.vector.reciprocal(rz1[:, 4 * c:4 * c + 4], zt[:, 4:8])

            # out projection + per-head 1/Z merge
            ost = wpool.tile([128, 4, D], bf16, name="ost", tag="ost", bufs=2)
            for i in range(4):
                t = 4 * c + i
                po0 = pp.tile([128, D], f32, name="po0", tag="px", bufs=2)
                nc.tensor.matmul(po0[:, :],
                                 lhsT=ctxT[0:DH, t * 128:(t + 1) * 128],
                                 rhs=wo_sb[0:DH, :], start=True, stop=True)
                po1 = pp.tile([128, D], f32, name="po1", tag="px", bufs=2)
                nc.tensor.matmul(po1[:, :],
                                 lhsT=ctxT[64:64 + DH, t * 128:(t + 1) * 128],
                                 rhs=wo_sb[64:64 + DH, :],
                                 start=True, stop=True)
                tm = wpool.tile([128, D], f32, name="tm", tag="tm", bufs=2)
                nc.vector.tensor_scalar_mul(tm[:, :], po0[:, :], rz0[:, t:t + 1])
                nc.vector.scalar_tensor_tensor(
                    ost[:, i, :], po1[:, :], rz1[:, t:t + 1], tm[:, :],
                    op0=ALU.mult, op1=ALU.add)
            nc.sync.dma_start(
                out=bass.AP(out, c * CQ * D, [[D, 128], [128 * D, 4], [1, D]]),
                in_=ost[:, :, :])

    return nc


def _shard_inputs(query, key, value, pos_emb, Wq, bq, Wk, bk, Wv, bv, Wp, Wo, bo):
    in_maps = []
    xt = {}
    for b in range(B):
        xt[("q", b)] = np.ascontiguousarray(query[b].T).astype(_BF16)
        xt[("k", b)] = np.ascontiguousarray(key[b].T).astype(_BF16)
        xt[("p", b)] = np.ascontiguousarray(pos_emb[b].T).astype(_BF16)
        xt[("v", b)] = np.ascontiguousarray(value[b].T).astype(_BF16)
    wq16 = (Wq.astype(np.float32) * SCALE).astype(_BF16)
    wk16, wp16, wv16, wo16 = (w.astype(_BF16) for w in (Wk, Wp, Wv, Wo))
    bq_s = bq.astype(np.float32) * SCALE
    ident = np.eye(128, dtype=np.float32).astype(_BF16)
    for c in range(NCORES):
        b, hp = c // 4, c % 4
        cs = slice(hp * HPC * DH, (hp + 1) * HPC * DH)
        in_maps.append({
            "xq_t": xt[("q", b)],
            "xk_t": xt[("k", b)],
            "xp_t": xt[("p", b)],
            "xv_t": xt[("v", b)],
            "wq": np.ascontiguousarray(wq16[:, cs]),
            "wk": np.ascontiguousarray(wk16[:, cs]),
            "wp": np.ascontiguousarray(wp16[:, cs]),
            "wv": np.ascontiguousarray(wv16[:, cs]),
            "wo": np.ascontiguousarray(wo16[cs, :]),
            "ident": ident,
            "bq": np.ascontiguousarray(bq_s[cs]).reshape(HPC * DH, 1),
            "bk": np.ascontiguousarray(bk[cs]).reshape(HPC * DH, 1).astype(np.float32),
        })
    return in_maps


def _unshard(results, Wo, bv, bo):
    const = (bv.astype(np.float32) @ Wo.astype(np.float32)) + bo.astype(np.float32)
    out = np.zeros((B, L, D), np.float32)
    for c in range(NCORES):
        out[c // 4] += results[c]["out"].astype(np.float32).reshape(L, D)
    out += const[None, None, :]
    return out


_CACHE = {}


def kernel(query, key, value, pos_emb, Wq, bq, Wk, bk, Wv, bv, Wp, Wo, bo,
           _want_profile=False):
    import sys
    if "/opt/trn_rl_repo" not in sys.path:
        sys.path.insert(0, "/opt/trn_rl_repo")
    from concourse.bass_utils import run_bass_kernel_spmd

    args = [np.asarray(a) for a in
            (query, key, value, pos_emb, Wq, bq, Wk, bk, Wv, bv, Wp, Wo, bo)]
    (query, key, value, pos_emb, Wq, bq, Wk, bk, Wv, bv, Wp, Wo, bo) = args

    if "nc" not in _CACHE:
        nc = build_nc()
        if not nc.is_finalized():
            nc.finalize()
        _CACHE["nc"] = nc
    nc = _CACHE["nc"]

    in_maps = _shard_inputs(query, key, value, pos_emb, Wq, bq, Wk, bk, Wv, bv,
                            Wp, Wo, bo)
    res = run_bass_kernel_spmd(nc, in_maps, list(range(NCORES)),
                               trace=_want_profile)
    out = _unshard(res.results, Wo, bv, bo)
    if _want_profile:
        return out, res
    return out


# revision 28
# speedup vs baseline: 1.1710x; 1.1710x over previous
"""Trainium2 Bass kernel for nn_RelativeMultiHeadAttention.

Full (unsharded) numpy inputs in, full output out. Internally shards across
8 NeuronCores: core c handles batch b = c//4 and head pair hp = c%4
(heads 2*hp, 2*hp+1).

v2: q-chunk software pipeline. The relative shift is q-local, so the kernel
processes queries in 4 chunks of 512:
  - pos scores stream to a padded DRAM buffer G (Transformer-XL shift becomes
    an overlapping-window read of G); shifted scores for chunk c need only
    G rows [512c, 512c+512], so the write->transposed-read roundtrip is
    pipelined chunk by chunk instead of a full barrier.
  - one batched xbar-transpose DMA per (chunk, head) pulls shifted_T
    [128, 16, 512] (out[p, kt, q] = G[row q, col 128*kt+p]).
  - content scores computed transposed; DVE/Pool add shifted, ACT exps to
    bf16 attn (no max-subtraction: |logit*scale| < ~4); A.V accumulates
    ctx_T[65, 512] per head with an appended ones-row giving Z = sum_k attn.
  - out projection per q-tile with per-partition 1/Z scaling; per-core
    partial written bf16; host sums partials + exact bv/bo correction.
SCALE is folded into Wq/bq on the host so logits come out pre-scaled.
"""

import numpy as np
import ml_dtypes

B, L, D, H = 2, 2048, 512, 8
DH = D // H            # 64
HPC = 2                # heads per core
NCORES = 8
SCALE = 1.0 / float(np.sqrt(D))
LQT = L // 128         # 16 q/k tiles of 128
NCH = 4                # q chunks
CQ = L // NCH          # 512 queries per chunk
GROWS = L + 1          # padded G row length (2049)
WARM = 7               # pos tiles computed before chunk 0

_BF16 = ml_dtypes.bfloat16


def build_nc():
    import concourse.bass as bass
    import concourse.mybir as mybir
    from concourse.bacc import Bacc
    from concourse.tile import TileContext
    from contextlib import ExitStack

    bf16 = mybir.dt.bfloat16
    f32 = mybir.dt.float32
    AF = mybir.ActivationFunctionType
    ALU = mybir.AluOpType

    nc = Bacc()

    xq = nc.declare_dram_parameter("xq_t", [D, L], bf16, isOutput=False)
    xk = nc.declare_dram_parameter("xk_t", [D, L], bf16, isOutput=False)
    xp = nc.declare_dram_parameter("xp_t", [D, L], bf16, isOutput=False)
    xv = nc.declare_dram_parameter("xv_t", [D, L], bf16, isOutput=False)
    wq = nc.declare_dram_parameter("wq", [D, HPC * DH], bf16, isOutput=False)
    wk = nc.declare_dram_parameter("wk", [D, HPC * DH], bf16, isOutput=False)
    wp = nc.declare_dram_parameter("wp", [D, HPC * DH], bf16, isOutput=False)
    wv = nc.declare_dram_parameter("wv", [D, HPC * DH], bf16, isOutput=False)
    wo = nc.declare_dram_parameter("wo", [HPC * DH, D], bf16, isOutput=False)
    ident = nc.declare_dram_parameter("ident", [128, 128], bf16, isOutput=False)
    bq = nc.declare_dram_parameter("bq", [HPC * DH, 1], f32, isOutput=False)
    bk = nc.declare_dram_parameter("bk", [HPC * DH, 1], f32, isOutput=False)
    out = nc.declare_dram_parameter("out", [L * D], bf16, isOutput=True)

    # per-(chunk, head) G buffers: 513 rows each (chunk boundary row is
    # written twice) so the transposed read of chunk c depends only on its
    # own chunk's writers (DRAM dep tracking is per-tensor).
    gs = [[nc.dram_tensor(f"g{h}_{c}", [(513 if c < NCH - 1 else 512) * GROWS],
                          bf16) for c in range(NCH)] for h in range(HPC)]

    with TileContext(nc) as tc, ExitStack() as top:
        # ---------------- pools ----------------
        persist = top.enter_context(tc.tile_pool(name="persist", bufs=1))

        def ptile(shape, dtype, name):
            return persist.tile(shape, dtype, name=name, tag=name)

        qT = ptile([128, L], bf16, "qT")
        kT = ptile([128, L], bf16, "kT")
        pT = ptile([128, L], bf16, "pT")
        vaug = ptile([128, LQT, 2 * (DH + 1)], bf16, "vaug")
        wo_sb = ptile([128, D], bf16, "wo_sb")
        ctxT = ptile([128, L], bf16, "ctxT")
        bq_sb = ptile([128, 1], f32, "bq_sb")
        bk_sb = ptile([128, 1], f32, "bk_sb")
        ones_sb = ptile([1, 1], f32, "ones_sb")
        zrow0 = ptile([1, L], f32, "zrow0")
        zrow1 = ptile([1, L], f32, "zrow1")
        rz0 = ptile([128, LQT], f32, "rz0")
        rz1 = ptile([128, LQT], f32, "rz1")
        id_sb = ptile([128, 128], bf16, "id_sb")
        zrows = [zrow0, zrow1]
        rzs = [rz0, rz1]

        # single PSUM pool, 8 banks total across tags:
        # score(2) + pos(2) + po(2) + av0(1) + av1(1)
        pp = top.enter_context(tc.tile_pool(name="pp", bufs=1, space="PSUM"))

        nc.vector.memset(ones_sb[:, :], 1.0)
        nc.vector.memset(vaug[:, :, DH:DH + 1], 1.0)
        nc.vector.memset(vaug[:, :, 2 * DH + 1:2 * DH + 2], 1.0)
        nc.scalar.dma_start(out=wo_sb[:, :], in_=wo[:, :])
        nc.scalar.dma_start(out=bq_sb[:, :], in_=bq[:, :])
        nc.scalar.dma_start(out=bk_sb[:, :], in_=bk[:, :])
        nc.scalar.dma_start(out=id_sb[:, :], in_=ident[:, :])

        # ---------------- phase 1: projections ----------------
        with ExitStack() as p1:
            inpool = p1.enter_context(tc.tile_pool(name="inpool", bufs=1))
            x_sbs = {}
            w_sbs = {}
            for name, src in (("p", xp), ("q", xq), ("k", xk), ("v", xv)):
                t = inpool.tile([128, 4, L], bf16, name=f"x_{name}", tag=f"x_{name}")
                nc.sync.dma_start(
                    out=t[:, :, :],
                    in_=src[:, :].rearrange("(c p) m -> p c m", p=128))
                x_sbs[name] = t
            for name, src in (("p", wp), ("q", wq), ("k", wk), ("v", wv)):
                t = inpool.tile([128, 4, HPC * DH], bf16, name=f"w_{name}",
                                tag=f"w_{name}")
                nc.scalar.dma_start(
                    out=t[:, :, :],
                    in_=src[:, :].rearrange("(c p) m -> p c m", p=128))
                w_sbs[name] = t

            # pT/qT/kT: [128 (2 heads x 64ch), L], bias folded; q pre-scaled
            # on host (SCALE into wq/bq) so scores come out pre-scaled.
            for name, dst, bias in (
                    ("p", pT, None),
                    ("q", qT, bq_sb),
                    ("k", kT, bk_sb)):
                xs, ws = x_sbs[name], w_sbs[name]
                for n in range(4):
                    ps = pp.tile([128, 512], f32, name="ps_pj", tag="score",
                                 bufs=2)
                    for ck in range(4):
                        nc.tensor.matmul(
                            ps[:, :], lhsT=ws[:, ck, :],
                            rhs=xs[:, ck, n * 512:(n + 1) * 512],
                            start=(ck == 0), stop=(ck == 3))
                    o = dst[:, n * 512:(n + 1) * 512]
                    if name == "k":
                        nc.vector.tensor_scalar_add(o, ps[:, :], bias[:, 0:1])
                    elif name == "q":
                        nc.scalar.add(o, ps[:, :], bias[:, 0:1])
                    else:
                        nc.scalar.copy(o, ps[:, :])

            # v natural: [L, 128ch] -> vaug [128, t, [v0|1|v1|1]]
            xs, ws = x_sbs["v"], w_sbs["v"]
            for t in range(LQT):
                ps = pp.tile([128, 512], f32, name="ps_v", tag="px", bufs=2)
                for ck in range(4):
                    nc.tensor.matmul(
                        ps[:, 0:128], lhsT=xs[:, ck, t * 128:(t + 1) * 128],
                        rhs=ws[:, ck, :], start=(ck == 0), stop=(ck == 3))
                nc.vector.tensor_copy(vaug[:, t, 0:DH], ps[:, 0:DH])
                nc.vector.tensor_copy(vaug[:, t, DH + 1:2 * DH + 1],
                                      ps[:, DH:2 * DH])

        # ---------------- phase 2: pipelined pos/shift/attn ----------------
        stpool = top.enter_context(tc.tile_pool(name="stpool", bufs=4))
        shpool = top.enter_context(tc.tile_pool(name="shpool", bufs=3))
        wpool = top.enter_context(tc.tile_pool(name="wpool", bufs=1))

        stage_tiles = {}
        sh_tiles = {}
        def stage_for(t, h):
            st = stpool.tile([128, GROWS], bf16, name=f"st{t}_{h}", tag="st")
            stage_tiles[(t, h)] = st
            nc.gpsimd.memset(st[:, L:GROWS], 0.0)
            return st

        def gwrite(t, h):
            st = stage_tiles[(t, h)]
            cc = t // 4
            nc.gpsimd.dma_start(
                out=bass.AP(gs[h][cc], (t % 4) * 128 * GROWS,
                            [[GROWS, 128], [1, GROWS]]),
                in_=st[:, :])
            if t % 4 == 0 and t > 0:
                # chunk boundary row: first row of this tile is also the
                # last (513th) row of the previous chunk's G
                nc.gpsimd.dma_start(
                    out=bass.AP(gs[h][cc - 1], 512 * GROWS,
                                [[GROWS, 1], [1, GROWS]]),
                    in_=st[0:1, :])
            del stage_tiles[(t, h)]

        def pos_piece(t, h, q4, warm):
            hb = h * DH
            if q4 == 0:
                stage_for(t, h)
            st = stage_tiles[(t, h)]
            ps = pp.tile([128, 512], f32, name="ps_pos", tag="px", bufs=2)
            nc.tensor.matmul(
                ps[:, :], lhsT=qT[hb:hb + DH, t * 128:(t + 1) * 128],
                rhs=pT[hb:hb + DH, q4 * 512:(q4 + 1) * 512],
                start=True, stop=True)
            o = st[:, q4 * 512:(q4 + 1) * 512]
            nc.vector.tensor_copy(o, ps[:, :])
            if q4 == 3:
                gwrite(t, h)

        def pos_tile_pair(t, h, half, eng):
            # warmup fast path: both halves of a pos tile through the (idle)
            # 2-bank score tag, one big copy, alternating copy engines
            hb = h * DH
            if half == 0:
                stage_for(t, h)
            st = stage_tiles[(t, h)]
            ps = pp.tile([128, 1024], f32, name="ps_posw", tag="score",
                         bufs=2)
            for q in range(2):
                q4 = 2 * half + q
                nc.tensor.matmul(
                    ps[:, q * 512:(q + 1) * 512],
                    lhsT=qT[hb:hb + DH, t * 128:(t + 1) * 128],
                    rhs=pT[hb:hb + DH, q4 * 512:(q4 + 1) * 512],
                    start=True, stop=True)
            o = st[:, half * 1024:(half + 1) * 1024]
            if eng == 0:
                nc.scalar.copy(o, ps[:, :])
            else:
                nc.vector.tensor_copy(o, ps[:, :])
            if half == 1:
                gwrite(t, h)

        def issue_T(c):
            for h in range(HPC):
                sht = shpool.tile([128, LQT, CQ], bf16, name=f"sh{c}_{h}",
                                  tag=f"sh{h}")
                sh_tiles[(c, h)] = sht
                for half in range(2):
                    nc.sync.dma_start(
                        out=sht[:, 8 * half:8 * half + 8, :],
                        in_=bass.AP(gs[h][c],
                                    (L - 1) - CQ * c + 1024 * half,
                                    [[L, CQ], [1, 1024]]),
                        transpose=True)

        # warmup: tiles 0..WARM-1 as dense pairs; T(0) issued as soon as its
        # writers (tiles 0-3 + boundary row of tile 4) are emitted
        ei = 0
        for t in range(WARM):
            for h in range(HPC):
                for half in range(2):
                    pos_tile_pair(t, h, half, ei % 2)
                    ei += 1
            if t == 4:
                issue_T(0)
        rest = [(t, h, q4) for t in range(WARM, LQT) for h in range(HPC)
                for q4 in range(4)]
        ri = 0

        def emit_av(c, h, pk, pattn, stop):
            nc.tensor.matmul(
                avs[h][:, :],
                lhsT=vaug[:, 2 * pk, h * (DH + 1):(h + 1) * (DH + 1)],
                rhs=pattn[:, 0:CQ], start=(pk == 0), stop=False)
            nc.tensor.matmul(
                avs[h][:, :],
                lhsT=vaug[:, 2 * pk + 1, h * (DH + 1):(h + 1) * (DH + 1)],
                rhs=pattn[:, CQ:2 * CQ], start=False, stop=stop)

        def outproj_steps(c):
            # generator of deferred out-projection steps for chunk c,
            # interleaved into the next chunk's PE stream
            def s_z():
                zt = pp.tile([128, 512], f32, name="zt", tag="px", bufs=2)
                for i in range(4):
                    t = 4 * c + i
                    nc.tensor.matmul(
                        zt[:, i:i + 1],
                        lhsT=zrow0[0:1, t * 128:(t + 1) * 128],
                        rhs=ones_sb[0:1, 0:1], start=True, stop=True)
                    nc.tensor.matmul(
                        zt[:, 4 + i:5 + i],
                        lhsT=zrow1[0:1, t * 128:(t + 1) * 128],
                        rhs=ones_sb[0:1, 0:1], start=True, stop=True)
                nc.vector.reciprocal(rz0[:, 4 * c:4 * c + 4], zt[:, 0:4])
                nc.vector.reciprocal(rz1[:, 4 * c:4 * c + 4], zt[:, 4:8])
                ost = wpool.tile([128, 4, D], bf16, name="ost", tag="ost",
                                 bufs=2)
                state["ost"] = ost
            yield s_z
            for i in range(4):
                def s_po(i=i):
                    t = 4 * c + i
                    ost = state["ost"]
                    po0 = pp.tile([128, D], f32, name="po0", tag="px", bufs=2)
                    nc.tensor.matmul(
                        po0[:, :], lhsT=ctxT[0:DH, t * 128:(t + 1) * 128],
                        rhs=wo_sb[0:DH, :], start=True, stop=True)
                    po1 = pp.tile([128, D], f32, name="po1", tag="px", bufs=2)
                    nc.tensor.matmul(
                        po1[:, :],
                        lhsT=ctxT[64:64 + DH, t * 128:(t + 1) * 128],
                        rhs=wo_sb[64:64 + DH, :], start=True, stop=True)
                    tm = wpool.tile([128, D], f32, name="tm", tag="tm",
                                    bufs=2)
                    nc.vector.tensor_scalar_mul(tm[:, :], po0[:, :],
                                                rz0[:, t:t + 1])
                    nc.vector.scalar_tensor_tensor(
                        ost[:, i, :], po1[:, :], rz1[:, t:t + 1], tm[:, :],
                        op0=ALU.mult, op1=ALU.add)
                yield s_po
            def s_dma():
                nc.sync.dma_start(
                    out=bass.AP(out, c * CQ * D,
                                [[D, 128], [128 * D, 4], [1, D]]),
                    in_=state["ost"][:, :, :])
            yield s_dma

        state = {}
        avs = [None, None]
        NKP = LQT // 2  # kt pairs
        deferred = None
        for c in range(NCH):
            pend = {0: [], 1: []}  # attn pairs awaiting A.V (2-pair delay)
            for ktp in range(NKP):
                if ktp == 4 and c + 1 < NCH:
                    issue_T(c + 1)
                if deferred is not None and 1 <= ktp <= 6:
                    nxt = next(deferred, None)
                    if nxt is not None:
                        nxt()
                k0, k1 = 2 * ktp, 2 * ktp + 1
                for h in range(HPC):
                    for _ in range(2):
                        if ri < len(rest):
                            pos_piece(*rest[ri], warm=False)
                            ri += 1
                    hb = h * DH
                    # kt-pair content into a 2-bank tile, shifted scores
                    # accumulated by PE identity matmuls, one big exp
                    ct = pp.tile([128, 2 * CQ], f32, name="ps_ct", tag="score",
                                 bufs=2)
                    nc.tensor.matmul(
                        ct[:, 0:CQ],
                        lhsT=kT[hb:hb + DH, k0 * 128:(k0 + 1) * 128],
                        rhs=qT[hb:hb + DH, c * CQ:(c + 1) * CQ],
                        start=True, stop=False)
                    nc.tensor.matmul(
                        ct[:, CQ:2 * CQ],
                        lhsT=kT[hb:hb + DH, k1 * 128:(k1 + 1) * 128],
                        rhs=qT[hb:hb + DH, c * CQ:(c + 1) * CQ],
                        start=True, stop=False)
                    nc.tensor.matmul(ct[:, 0:CQ], lhsT=id_sb[:, :],
                                     rhs=sh_tiles[(c, h)][:, k0, :],
                                     start=False, stop=True)
                    nc.tensor.matmul(ct[:, CQ:2 * CQ], lhsT=id_sb[:, :],
                                     rhs=sh_tiles[(c, h)][:, k1, :],
                                     start=False, stop=True)
                    # A.V lags two pairs behind so exp is never on the
                    # critical path of the PE stream
                    if len(pend[h]) >= 2:
                        pk, pattn = pend[h].pop(0)
                        emit_av(c, h, pk, pattn, stop=False)
                    attn = wpool.tile([128, 2 * CQ], bf16, name="attn",
                                      tag="attn", bufs=6)
                    nc.scalar.activation(attn[:, :], ct[:, :], AF.Exp,
                                         bias=0.0, scale=1.0)
                    if ktp == 0:
                        avs[h] = pp.tile([DH + 1, CQ], f32, name=f"av{h}",
                                         tag=f"av{h}", bufs=1)
                    pend[h].append((ktp, attn))
            for h in range(HPC):
                while pend[h]:
                    pk, pattn = pend[h].pop(0)
                    emit_av(c, h, pk, pattn, stop=(not pend[h]))
            del sh_tiles[(c, 0)], sh_tiles[(c, 1)]

            # ctx rows + Z row out of the A.V accumulators
            for h in range(HPC):
                nc.vector.tensor_copy(
                    ctxT[64 * h:64 * h + DH, c * CQ:(c + 1) * CQ],
                    avs[h][0:DH, :])
                nc.scalar.copy(zrows[h][0:1, c * CQ:(c + 1) * CQ],
                               avs[h][DH:DH + 1, :])
            deferred = outproj_steps(c)

        # flush the last chunk's out projection
        for step in deferred:
            step()

    return nc


def _shard_inputs(query, key, value, pos_emb, Wq, bq, Wk, bk, Wv, bv, Wp, Wo, bo):
    in_maps = []
    xt = {}
    for b in range(B):
        xt[("q", b)] = np.ascontiguousarray(query[b].T).astype(_BF16)
        xt[("k", b)] = np.ascontiguousarray(key[b].T).astype(_BF16)
        xt[("p", b)] = np.ascontiguousarray(pos_emb[b].T).astype(_BF16)
        xt[("v", b)] = np.ascontiguousarray(value[b].T).astype(_BF16)
    wq16 = (Wq.astype(np.float32) * SCALE).astype(_BF16)
    wk16, wp16, wv16, wo16 = (w.astype(_BF16) for w in (Wk, Wp, Wv, Wo))
    bq_s = bq.astype(np.float32) * SCALE
    ident = np.eye(128, dtype=np.float32).astype(_BF16)
    for c in range(NCORES):
        b, hp = c // 4, c % 4
        cs = slice(hp * HPC * DH, (hp + 1) * HPC * DH)
        in_maps.append({
            "xq_t": xt[("q", b)],
            "xk_t": xt[("k", b)],
            "xp_t": xt[("p", b)],
            "xv_t": xt[("v", b)],
            "wq": np.ascontiguousarray(wq16[:, cs]),
            "wk": np.ascontiguousarray(wk16[:, cs]),
            "wp": np.ascontiguousarray(wp16[:, cs]),
            "wv": np.ascontiguousarray(wv16[:, cs]),
            "wo": np.ascontiguousarray(wo16[cs, :]),
            "ident": ident,
            "bq": np.ascontiguousarray(bq_s[cs]).reshape(HPC * DH, 1),
            "bk": np.ascontiguousarray(bk[cs]).reshape(HPC * DH, 1).astype(np.float32),
        })
    return in_maps


def _unshard(results, Wo, bv, bo):
    const = (bv.astype(np.float32) @ Wo.astype(np.float32)) + bo.astype(np.float32)
    out = np.zeros((B, L, D), np.float32)
    for c in range(NCORES):
        out[c // 4] += results[c]["out"].astype(np.float32).reshape(L, D)
    out += const[None, None, :]
    return out


_CACHE = {}


def kernel(query, key, value, pos_emb, Wq, bq, Wk, bk, Wv, bv, Wp, Wo, bo,
           _want_profile=False):
    import sys
    if "/opt/trn_rl_repo" not in sys.path:
        sys.path.insert(0, "/opt/trn_rl_repo")
    from concourse.bass_utils import run_bass_kernel_spmd

    args = [np.asarray(a) for a in
            (query, key, value, pos_emb, Wq, bq, Wk, bk, Wv, bv, Wp, Wo, bo)]
    (query, key, value, pos_emb, Wq, bq, Wk, bk, Wv, bv, Wp, Wo, bo) = args

    if "nc" not in _CACHE:
        nc = build_nc()
        if not nc.is_finalized():
            nc.finalize()
        _CACHE["nc"] = nc
    nc = _CACHE["nc"]

    in_maps = _shard_inputs(query, key, value, pos_emb, Wq, bq, Wk, bk, Wv, bv,
                            Wp, Wo, bo)
    res = run_bass_kernel_spmd(nc, in_maps, list(range(NCORES)),
                               trace=_want_profile)
    out = _unshard(res.results, Wo, bv, bo)
    if _want_profile:
        return out, res
    return out
